# revision 1
# baseline (speedup 1.0000x reference)
"""Trainium2 Bass kernel for nn_CrossAttentionLayer (4-stream cross attention).

kernel(**inputs) takes FULL unsharded inputs (keyed as in setup_inputs) and
returns the full output (tuple of 4 arrays, like the reference). Batch (8) is
sharded 1 element per NeuronCore across 8 cores (pure data parallel).

Geometry per core, with C=512, L=256, H=W=64, N=4096:
  The reference's raw .view on the [L,H,W] conv output re-interprets it as
  [H,W,L]; since L=4*64, token t=(l,b) (l=0..255 conv channel, b=0..15)
  has feature vector y[l, b*256 : (b+1)*256] -- a CONTIGUOUS 256-pixel run
  of row l. Output pixel n = l*16 + b corresponds 1:1 to token (l,b).

  So in the natural [L(part), N(free)] layout, a [128, 256] slice is 128
  tokens x 256 features: attention scalars (sim, softmax, attn) are
  per-partition values -- no cross-partition work anywhere.

Pipeline per core:
  phase1 (per 512-px chunk, per stream): DMA x chunk; fp32r matmuls for
    k|q (BN scale folded into weights host-side) and v; ACT evacuates PSUM
    with relu+bias (k,q) / bias (v) as bf16 chunk tiles.
  attention (per chunk): sim via fused scalar_tensor_tensor per (pair,
    l-tile, b); one-shot strided-AP softmax over the 4 k-streams; ctx via
    scalar_tensor_tensor chains with per-partition attn scalars; PE
    transposes ctx into rhs[f, b*256+l] (b-major, contiguous copies).
  phase2 (per 512-token block): up-projection matmul reading rhs through a
    strided (l-outer, b-inner) AP so PSUM columns land in output-pixel
    order, plus an fp32r identity matmul adding the residual x; bias via
    copy-with-bias; batched 1 MiB output DMA.
"""

import numpy as np

import concourse.bass as bass
import concourse.bacc as bacc
import concourse.mybir as mybir
from concourse.tile import TileContext
from concourse.bass_utils import run_bass_kernel_spmd

B, C, L, HW = 8, 512, 256, 64
N = HW * HW              # 4096 pixels
F = 256                  # token feature length (= N // 16)
NB = N // F              # 16 b-blocks
EPS = 1e-5
NCORES = 8
CHUNK = 512              # pixel chunk (2 b-blocks)
NCHUNKS = N // CHUNK     # 8
CC = C // 128            # 4 contraction chunks
LT = L // 128            # 2 l-tiles

FP32 = mybir.dt.float32
FP32R = mybir.dt.float32r
BF16 = mybir.dt.bfloat16
AF = mybir.ActivationFunctionType
ALU = mybir.AluOpType

_cached = {}


def _build_program(loop_iters=None):
    nc = bacc.Bacc("TRN2", target_bir_lowering=False, debug=False)

    xs = [nc.declare_dram_parameter(f"x{s}", [C, N], FP32R, isOutput=False)
          for s in range(4)]
    # host-prearranged weight images (exact SBUF layouts)
    wkq_d = nc.declare_dram_parameter("wkq", [128, 4 * 4 * 4 * 128], FP32R, isOutput=False)
    wd_d = nc.declare_dram_parameter("wd", [128, 4 * 2 * 4 * 128], FP32R, isOutput=False)
    wu_d = nc.declare_dram_parameter("wu", [128, 4 * 2 * 4 * 128], BF16, isOutput=False)
    bkq_d = nc.declare_dram_parameter("bkq", [128, 16], FP32, isOutput=False)
    bd_d = nc.declare_dram_parameter("bd", [128, 8], FP32, isOutput=False)
    bu_d = nc.declare_dram_parameter("bu", [128, 16], FP32, isOutput=False)
    id_d = nc.declare_dram_parameter("ident", [128, 128], FP32R, isOutput=False)
    idb_d = nc.declare_dram_parameter("identb", [128, 128], BF16, isOutput=False)
    os_ = [nc.declare_dram_parameter(f"o{s}", [C, N], FP32, isOutput=True)
           for s in range(4)]

    with TileContext(nc) as tc:
        with (
            tc.tile_pool(name="wpool", bufs=1) as wpool,
            tc.tile_pool(name="xpool", bufs=2) as xpool,
            tc.tile_pool(name="kqvp", bufs=1) as kqvp,
            tc.tile_pool(name="attp", bufs=1) as attp,
            tc.tile_pool(name="rhsp", bufs=1) as rhsp,
            tc.tile_pool(name="outp", bufs=1) as outp,
            tc.tile_pool(name="ps_c", bufs=4, space="PSUM") as ps_c,
            tc.tile_pool(name="ps_t", bufs=2, space="PSUM") as ps_t,
            tc.tile_pool(name="ps_up", bufs=2, space="PSUM") as ps_up,
        ):
            # ---- weights ----
            wkq = wpool.tile([128, 16, 4, 128], FP32R)   # [c, (s,mc), j, m]
            nc.sync.dma_start(out=wkq[:], in_=wkq_d.ap().rearrange(
                "p (a j m) -> p a j m", a=16, j=4))
            wd = wpool.tile([128, 8, 4, 128], FP32R)     # [c, (s,lt), j, m]
            nc.sync.dma_start(out=wd[:], in_=wd_d.ap().rearrange(
                "p (a j m) -> p a j m", a=8, j=4))
            wu = wpool.tile([128, 8, 4, 128], BF16)     # [f, (s,fh), j, c]
            nc.sync.dma_start(out=wu[:], in_=wu_d.ap().rearrange(
                "p (a j m) -> p a j m", a=8, j=4))
            bkq = wpool.tile([128, 16], FP32)
            nc.sync.dma_start(out=bkq[:], in_=bkq_d.ap())
            bd = wpool.tile([128, 8], FP32)
            nc.sync.dma_start(out=bd[:], in_=bd_d.ap())
            bu = wpool.tile([128, 16], FP32)
            nc.sync.dma_start(out=bu[:], in_=bu_d.ap())
            ident = wpool.tile([128, 128], FP32R)
            nc.sync.dma_start(out=ident[:], in_=id_d.ap())
            identb = wpool.tile([128, 128], BF16)
            nc.sync.dma_start(out=identb[:], in_=idb_d.ap())

            # rhs: transposed ctx, per stream 2 f-half tiles; column layout
            # b*256 + lt*128 + l_local (b-major -> contiguous writes).
            rhs = rhsp.tile([128, 4, 2, N], BF16)  # [f_local, s, fh, col]

            def _body():
                for ci in range(NCHUNKS):
                    n0 = ci * CHUNK
                    kch, qch, vch = [], [], []
                    for s in range(4):
                        xt = xpool.tile([128, CC, CHUNK], FP32R, tag="x", name="xt")
                        nc.sync.dma_start(
                            out=xt[:],
                            in_=xs[s].ap().rearrange("(j p) n -> p j n", p=128)[:, :, n0:n0 + CHUNK])
                        kc = kqvp.tile([128, LT, CHUNK], BF16, tag=f"k{s}", name=f"kc{s}")
                        qc = kqvp.tile([128, LT, CHUNK], BF16, tag=f"q{s}", name=f"qc{s}")
                        vc = kqvp.tile([128, LT, CHUNK], BF16, tag=f"v{s}", name=f"vc{s}")
                        # k|q: mc 0,1 = k l-tiles; 2,3 = q l-tiles
                        for mc in range(4):
                            pcv = ps_c.tile([128, CHUNK], FP32, tag="conv", name="pcv")
                            for j in range(CC):
                                nc.tensor.matmul(
                                    out=pcv[:], lhsT=wkq[:, s * 4 + mc, j, :],
                                    rhs=xt[:, j, :],
                                    start=(j == 0), stop=(j == CC - 1))
                            dst = (kc if mc < 2 else qc)[:, mc % 2, :]
                            nc.scalar.activation(
                                out=dst, in_=pcv[:], func=AF.Relu,
                                bias=bkq[:, s * 4 + mc:s * 4 + mc + 1], scale=1.0)
                        for mc in range(2):
                            pcv = ps_c.tile([128, CHUNK], FP32, tag="conv", name="pcv2")
                            for j in range(CC):
                                nc.tensor.matmul(
                                    out=pcv[:], lhsT=wd[:, s * 2 + mc, j, :],
                                    rhs=xt[:, j, :],
                                    start=(j == 0), stop=(j == CC - 1))
                            nc.scalar.activation(
                                out=vc[:, mc, :], in_=pcv[:], func=AF.Identity,
                                bias=bd[:, s * 2 + mc:s * 2 + mc + 1], scale=1.0)
                        kch.append(kc)
                        qch.append(qc)
                        vch.append(vc)

                    # ---- attention for this chunk (2 b-blocks) ----
                    # sims[l_local, s, s', lt, b] fp32
                    sims = attp.tile([128, 4, 4, LT, 2], FP32, tag="sims", name="sims")
                    scr = attp.tile([128, F], BF16, tag="scr", name="scr")
                    for s in range(4):
                        for s2 in range(4):
                            for lt in range(LT):
                                for b in range(2):
                                    nc.vector.scalar_tensor_tensor(
                                        out=scr[:],
                                        in0=qch[s][:, lt, b * F:(b + 1) * F],
                                        scalar=0.0625,
                                        in1=kch[s2][:, lt, b * F:(b + 1) * F],
                                        op0=ALU.mult, op1=ALU.mult,
                                        accum_out=sims[:, s, s2, lt, b:b + 1])
                    # softmax over s' (axis 2): strided views
                    mx = attp.tile([128, 4, LT, 2], FP32, tag="mx", name="mx")
                    nc.vector.tensor_reduce(
                        out=mx[:], in_=sims.rearrange("p s t l b -> p s l b t"),
                        axis=mybir.AxisListType.X, op=ALU.max)
                    ex = attp.tile([128, 4, 4, LT, 2], FP32, tag="ex", name="ex")
                    nc.vector.tensor_tensor(
                        out=ex[:], in0=sims[:],
                        in1=mx.rearrange("p s l b -> p s () l b").broadcast_to((128, 4, 4, LT, 2)),
                        op=ALU.subtract)
                    nc.scalar.activation(out=ex[:], in_=ex[:], func=AF.Exp,
                                         bias=0.0, scale=1.0)
                    sm = attp.tile([128, 4, LT, 2], FP32, tag="sm", name="sm")
                    nc.vector.tensor_reduce(
                        out=sm[:], in_=ex.rearrange("p s t l b -> p s l b t"),
                        axis=mybir.AxisListType.X, op=ALU.add)
                    nc.vector.reciprocal(out=sm[:], in_=sm[:])
                    att = attp.tile([128, 4, 4, LT, 2], FP32, tag="att", name="att")
                    nc.vector.tensor_tensor(
                        out=att[:], in0=ex[:],
                        in1=sm.rearrange("p s l b -> p s () l b").broadcast_to((128, 4, 4, LT, 2)),
                        op=ALU.mult)

                    # ---- ctx + transpose into rhs ----
                    for s in range(4):
                        ctx = attp.tile([128, LT, CHUNK], BF16, tag="ctx", name="ctx")
                        for lt in range(LT):
                            for b in range(2):
                                sl = slice(b * F, (b + 1) * F)
                                nc.vector.tensor_scalar_mul(
                                    out=ctx[:, lt, sl], in0=vch[0][:, lt, sl],
                                    scalar1=att[:, s, 0, lt, b:b + 1])
                                for s2 in range(1, 4):
                                    nc.vector.scalar_tensor_tensor(
                                        out=ctx[:, lt, sl], in0=vch[s2][:, lt, sl],
                                        scalar=att[:, s, s2, lt, b:b + 1],
                                        in1=ctx[:, lt, sl],
                                        op0=ALU.mult, op1=ALU.add)
                        for lt in range(LT):
                            for b in range(2):
                                bg = 2 * ci + b   # global b index
                                for fh in range(2):
                                    pst = ps_t.tile([128, 128], BF16, tag="pst", name="pst")
                                    nc.tensor.transpose(
                                        out=pst[:],
                                        in_=ctx[:, lt, b * F + fh * 128: b * F + (fh + 1) * 128],
                                        identity=identb[:])
                                    dst = rhs[:, s, fh, bg * 256 + lt * 128: bg * 256 + (lt + 1) * 128]
                                    if (lt + b) % 2 == 0:
                                        nc.vector.tensor_copy(dst, pst[:])
                                    else:
                                        nc.scalar.copy(out=dst, in_=pst[:])

                # ================= phase 2 =================
                for s in range(4):
                    for nb in range(NCHUNKS):
                        n0 = nb * CHUNK
                        lt, lo = nb // 4, (nb % 4) * 32
                        xt = xpool.tile([128, CC, CHUNK], FP32R, tag="x", name="xt2")
                        nc.sync.dma_start(
                            out=xt[:],
                            in_=xs[s].ap().rearrange("(j p) n -> p j n", p=128)[:, :, n0:n0 + CHUNK])
                        ot = outp.tile([128, CC, CHUNK], FP32, tag="ot", name="ot")
                        for j in range(CC):
                            pup = ps_up.tile([128, CHUNK], FP32, tag="up", name="pup")
                            for fh in range(2):
                                # rhs columns gathered l-outer, b-inner so psum
                                # columns are output-pixel order n = l*16 + b
                                rap = rhs[:, s, fh, :].rearrange(
                                    "p (b q) -> p b q", q=256)[:, :, lt * 128 + lo: lt * 128 + lo + 32]
                                rap = rap.rearrange("p b l -> p l b")
                                nc.tensor.matmul(
                                    out=pup[:], lhsT=wu[:, s * 2 + fh, j, :],
                                    rhs=rap, start=(fh == 0), stop=False)
                            nc.tensor.matmul(
                                out=pup[:], lhsT=ident[:],
                                rhs=xt[:, j, :],
                                start=False, stop=True)
                            if j % 2 == 0:
                                nc.vector.tensor_scalar_add(
                                    out=ot[:, j, :], in0=pup[:],
                                    scalar1=bu[:, s * 4 + j:s * 4 + j + 1])
                            else:
                                nc.scalar.activation(
                                    out=ot[:, j, :], in_=pup[:], func=AF.Identity,
                                    bias=bu[:, s * 4 + j:s * 4 + j + 1], scale=1.0)
                        nc.sync.dma_start(
                            out=os_[s].ap().rearrange("(j p) n -> p j n", p=128)[:, :, n0:n0 + CHUNK],
                            in_=ot[:])

            if loop_iters is None:
                _body()
            else:
                with tc.For_i(0, loop_iters, 1):
                    _body()

    nc.compile()
    return nc


def _prep_weights(inputs):
    """Fold BN into conv weights host-side; produce exact SBUF images."""
    import ml_dtypes
    f32 = np.float32
    g = {k: np.asarray(v, f32) for k, v in inputs.items()}
    sk = g["gk"] / np.sqrt(g["vk"] + EPS)            # [4, L]
    sq = g["gq"] / np.sqrt(g["vq"] + EPS)
    Wk_f = g["Wk"] * sk[:, :, None]                  # [4, L, C]
    Wq_f = g["Wq"] * sq[:, :, None]
    bk_f = (g["bk"] - g["mk"]) * sk + g["betak"]     # [4, L]
    bq_f = (g["bq"] - g["mq"]) * sq + g["betaq"]

    # wkq image [c_local, (s, mc), j, m]: lhsT chunks of [Wk_f|Wq_f]^T
    wkq = np.zeros((128, 16, 4, 128), f32)
    wdv = np.zeros((128, 8, 4, 128), f32)
    wuv = np.zeros((128, 8, 4, 128), f32)
    for s in range(4):
        Wcat = np.concatenate([Wk_f[s], Wq_f[s]], axis=0)  # [512 (kq-l), C]
        for mc in range(4):
            for j in range(CC):
                wkq[:, s * 4 + mc, j, :] = \
                    Wcat[mc * 128:(mc + 1) * 128, j * 128:(j + 1) * 128].T
        for mc in range(2):
            for j in range(CC):
                wdv[:, s * 2 + mc, j, :] = \
                    g["Wd"][s][mc * 128:(mc + 1) * 128, j * 128:(j + 1) * 128].T
        # wu: lhsT[f, c] = Wu[s].T ; [f_local, (s, fh), j, c_local]
        WuT = g["Wu"][s].T                           # [L=256 (f), C]
        for fh in range(2):
            for j in range(CC):
                wuv[:, s * 2 + fh, j, :] = \
                    WuT[fh * 128:(fh + 1) * 128, j * 128:(j + 1) * 128]

    bkq = np.zeros((128, 16), f32)
    bdv = np.zeros((128, 8), f32)
    buv = np.zeros((128, 16), f32)
    for s in range(4):
        for mc in range(4):
            src = bk_f[s] if mc < 2 else bq_f[s]
            bkq[:, s * 4 + mc] = src[(mc % 2) * 128:(mc % 2) * 128 + 128]
        for mc in range(2):
            bdv[:, s * 2 + mc] = g["bd"][s][mc * 128:(mc + 1) * 128]
        for j in range(CC):
            buv[:, s * 4 + j] = g["bu"][s][j * 128:(j + 1) * 128]
    ident = np.eye(128, dtype=f32)
    return {
        "wkq": wkq.reshape(128, -1), "wd": wdv.reshape(128, -1),
        "wu": wuv.reshape(128, -1).astype(ml_dtypes.bfloat16),
        "bkq": bkq, "bd": bdv, "bu": buv,
        "ident": ident, "identb": ident.astype(ml_dtypes.bfloat16),
    }


def get_program(loop_iters=None):
    key = ("nc", loop_iters)
    if key not in _cached:
        _cached[key] = _build_program(loop_iters)
    return _cached[key]


def make_in_maps(inputs):
    w = _prep_weights(inputs)
    names = ("x_f", "x_g", "x_h", "x_t")
    xs = {nm: np.asarray(inputs[nm], np.float32).reshape(B, C, N) for nm in names}
    in_maps = []
    for b in range(B):
        m = dict(w)
        for s, nm in enumerate(names):
            m[f"x{s}"] = np.ascontiguousarray(xs[nm][b])
        in_maps.append(m)
    return in_maps


def kernel(**inputs):
    nc = get_program()
    in_maps = make_in_maps(inputs)
    res = run_bass_kernel_spmd(nc, in_maps, core_ids=list(range(NCORES)))
    outs = []
    for s in range(4):
        o = np.stack([res.results[b][f"o{s}"] for b in range(B)], axis=0)
        outs.append(o.reshape(B, C, HW, HW))
    return tuple(outs)



# revision 2
# speedup vs baseline: 2.3995x; 2.3995x over previous
"""Trainium2 Bass kernel for nn_CrossAttentionLayer (4-stream cross attention).

kernel(**inputs) takes FULL unsharded inputs (keyed as in setup_inputs) and
returns the full output (tuple of 4 arrays, like the reference). Batch (8) is
sharded 1 element per NeuronCore across 8 cores (pure data parallel).

Geometry per core, with C=512, L=256, H=W=64, N=4096:
  The reference's raw .view on the [L,H,W] conv output re-interprets it as
  [H,W,L]; since N=16*256, token t=(a,r) (a=0..255 conv channel, r=0..15)
  has feature vector y[a, r*256:(r+1)*256] -- a CONTIGUOUS 256-pixel run of
  row a. Output pixel n = a*16 + r corresponds 1:1 to token (a,r).

v2 vs baseline:
  - All HBM I/O in bf16 (x pre-cast + pre-tiled host-side into a partition-
    major DMA image, outputs bf16, un-tiled host-side). Halves DMA traffic.
  - All matmuls bf16 (same PE rate as fp32r at free-dim 512, half SBUF
    traffic). Residual added via bf16 identity matmul into the up PSUM.
  - kqv/x tiles multi-buffered so chunk i attention (DVE) overlaps chunk
    i+1 convs (PE) and DMA.
  - rhs (transposed ctx) stored [f, s, fh, a, r] so phase-2 matmul rhs is
    fully contiguous; phase-1 transpose-evac copies are strided instead
    (they run at 1x regardless due to fp32 PSUM source).
"""

import numpy as np

import concourse.bass as bass
import concourse.bacc as bacc
import concourse.mybir as mybir
from concourse.tile import TileContext
from concourse.bass_utils import run_bass_kernel_spmd

B, C, L, HW = 8, 512, 256, 64
N = HW * HW              # 4096 pixels
F = 256                  # token feature length (= N // 16)
NR = 16                  # r-blocks (tokens per conv row)
EPS = 1e-5
NCORES = 8
CHUNK = 512              # pixel chunk (2 r-blocks)
NCHUNKS = N // CHUNK     # 8
CC = C // 128            # 4 contraction chunks
LT = L // 128            # 2 l-tiles

FP32 = mybir.dt.float32
BF16 = mybir.dt.bfloat16
AF = mybir.ActivationFunctionType
ALU = mybir.AluOpType

_cached = {}


def _build_program(loop_iters=None):
    nc = bacc.Bacc("TRN2", target_bir_lowering=False, debug=False)

    xs = [nc.declare_dram_parameter(f"x{s}", [128, NCHUNKS * CC * CHUNK], BF16,
                                    isOutput=False)
          for s in range(4)]
    wkq_d = nc.declare_dram_parameter("wkq", [128, 16 * 4 * 128], BF16, isOutput=False)
    wd_d = nc.declare_dram_parameter("wd", [128, 8 * 4 * 128], BF16, isOutput=False)
    wu_d = nc.declare_dram_parameter("wu", [128, 8 * 4 * 128], BF16, isOutput=False)
    bkq_d = nc.declare_dram_parameter("bkq", [128, 16], FP32, isOutput=False)
    bd_d = nc.declare_dram_parameter("bd", [128, 8], FP32, isOutput=False)
    bu_d = nc.declare_dram_parameter("bu", [128, 16], FP32, isOutput=False)
    idb_d = nc.declare_dram_parameter("identb", [128, 128], BF16, isOutput=False)
    os_ = [nc.declare_dram_parameter(f"o{s}", [128, NCHUNKS * CC * CHUNK], BF16,
                                     isOutput=True)
           for s in range(4)]

    with TileContext(nc) as tc:
        with (
            tc.tile_pool(name="wpool", bufs=1) as wpool,
            tc.tile_pool(name="xpool", bufs=6) as xpool,
            tc.tile_pool(name="kqvp", bufs=2) as kqvp,
            tc.tile_pool(name="attp", bufs=2) as attp,
            tc.tile_pool(name="rhsp", bufs=1) as rhsp,
            tc.tile_pool(name="outp", bufs=3) as outp,
            tc.tile_pool(name="ps_c", bufs=4, space="PSUM") as ps_c,
            tc.tile_pool(name="ps_t", bufs=2, space="PSUM") as ps_t,
            tc.tile_pool(name="ps_up", bufs=2, space="PSUM") as ps_up,
        ):
            # ---- weights ----
            wkq = wpool.tile([128, 16, 4, 128], BF16)   # [c, (s,mc), j, m]
            nc.sync.dma_start(out=wkq[:], in_=wkq_d.ap().rearrange(
                "p (a j m) -> p a j m", a=16, j=4))
            wd = wpool.tile([128, 8, 4, 128], BF16)     # [c, (s,lt), j, m]
            nc.sync.dma_start(out=wd[:], in_=wd_d.ap().rearrange(
                "p (a j m) -> p a j m", a=8, j=4))
            wu = wpool.tile([128, 8, 4, 128], BF16)     # [f, (s,fh), j, c]
            nc.sync.dma_start(out=wu[:], in_=wu_d.ap().rearrange(
                "p (a j m) -> p a j m", a=8, j=4))
            bkq = wpool.tile([128, 16], FP32)
            nc.sync.dma_start(out=bkq[:], in_=bkq_d.ap())
            bd = wpool.tile([128, 8], FP32)
            nc.sync.dma_start(out=bd[:], in_=bd_d.ap())
            bu = wpool.tile([128, 16], FP32)
            nc.sync.dma_start(out=bu[:], in_=bu_d.ap())
            identb = wpool.tile([128, 128], BF16)
            nc.sync.dma_start(out=identb[:], in_=idb_d.ap())

            # rhs: transposed ctx; [f_local, s, fh, a, r] so phase-2 reads
            # (a-major, r-minor) = output-pixel order contiguously.
            rhs = rhsp.tile([128, 4, 2, 256, NR], BF16)

            def _body():
                for ci in range(NCHUNKS):
                    kch, qch, vch = [], [], []
                    for s in range(4):
                        xt = xpool.tile([128, CC, CHUNK], BF16, tag="x", name="xt")
                        nc.sync.dma_start(
                            out=xt[:],
                            in_=xs[s].ap().rearrange(
                                "p (c j n) -> p c j n", c=NCHUNKS, j=CC)[:, ci])
                        kc = kqvp.tile([128, LT, CHUNK], BF16, tag=f"k{s}", name=f"kc{s}")
                        qc = kqvp.tile([128, LT, CHUNK], BF16, tag=f"q{s}", name=f"qc{s}")
                        vc = kqvp.tile([128, LT, CHUNK], BF16, tag=f"v{s}", name=f"vc{s}")
                        # k|q: mc 0,1 = k l-tiles; 2,3 = q l-tiles
                        for mc in range(4):
                            pcv = ps_c.tile([128, CHUNK], FP32, tag="conv", name="pcv")
                            for j in range(CC):
                                nc.tensor.matmul(
                                    out=pcv[:], lhsT=wkq[:, s * 4 + mc, j, :],
                                    rhs=xt[:, j, :],
                                    start=(j == 0), stop=(j == CC - 1))
                            dst = (kc if mc < 2 else qc)[:, mc % 2, :]
                            nc.scalar.activation(
                                out=dst, in_=pcv[:], func=AF.Relu,
                                bias=bkq[:, s * 4 + mc:s * 4 + mc + 1], scale=1.0)
                        for mc in range(2):
                            pcv = ps_c.tile([128, CHUNK], FP32, tag="conv", name="pcv2")
                            for j in range(CC):
                                nc.tensor.matmul(
                                    out=pcv[:], lhsT=wd[:, s * 2 + mc, j, :],
                                    rhs=xt[:, j, :],
                                    start=(j == 0), stop=(j == CC - 1))
                            nc.scalar.activation(
                                out=vc[:, mc, :], in_=pcv[:], func=AF.Identity,
                                bias=bd[:, s * 2 + mc:s * 2 + mc + 1], scale=1.0)
                        kch.append(kc)
                        qch.append(qc)
                        vch.append(vc)

                    # ---- attention for this chunk (2 r-blocks) ----
                    sims = attp.tile([128, 4, 4, LT, 2], FP32, tag="sims", name="sims")
                    scr = attp.tile([128, F], BF16, tag="scr", name="scr")
                    for s in range(4):
                        for s2 in range(4):
                            for lt in range(LT):
                                for r in range(2):
                                    nc.vector.scalar_tensor_tensor(
                                        out=scr[:],
                                        in0=qch[s][:, lt, r * F:(r + 1) * F],
                                        scalar=0.0625,
                                        in1=kch[s2][:, lt, r * F:(r + 1) * F],
                                        op0=ALU.mult, op1=ALU.mult,
                                        accum_out=sims[:, s, s2, lt, r:r + 1])
                    # softmax over s' (axis 2): strided views
                    mx = attp.tile([128, 4, LT, 2], FP32, tag="mx", name="mx")
                    nc.vector.tensor_reduce(
                        out=mx[:], in_=sims.rearrange("p s t l b -> p s l b t"),
                        axis=mybir.AxisListType.X, op=ALU.max)
                    ex = attp.tile([128, 4, 4, LT, 2], FP32, tag="ex", name="ex")
                    nc.vector.tensor_tensor(
                        out=ex[:], in0=sims[:],
                        in1=mx.rearrange("p s l b -> p s () l b").broadcast_to((128, 4, 4, LT, 2)),
                        op=ALU.subtract)
                    nc.scalar.activation(out=ex[:], in_=ex[:], func=AF.Exp,
                                         bias=0.0, scale=1.0)
                    sm = attp.tile([128, 4, LT, 2], FP32, tag="sm", name="sm")
                    nc.vector.tensor_reduce(
                        out=sm[:], in_=ex.rearrange("p s t l b -> p s l b t"),
                        axis=mybir.AxisListType.X, op=ALU.add)
                    nc.vector.reciprocal(out=sm[:], in_=sm[:])
                    att = attp.tile([128, 4, 4, LT, 2], FP32, tag="att", name="att")
                    nc.vector.tensor_tensor(
                        out=att[:], in0=ex[:],
                        in1=sm.rearrange("p s l b -> p s () l b").broadcast_to((128, 4, 4, LT, 2)),
                        op=ALU.mult)

                    # ---- ctx + transpose into rhs ----
                    for s in range(4):
                        ctx = attp.tile([128, LT, CHUNK], BF16, tag="ctx", name="ctx")
                        for lt in range(LT):
                            for r in range(2):
                                sl = slice(r * F, (r + 1) * F)
                                nc.vector.tensor_scalar_mul(
                                    out=ctx[:, lt, sl], in0=vch[0][:, lt, sl],
                                    scalar1=att[:, s, 0, lt, r:r + 1])
                                for s2 in range(1, 4):
                                    nc.vector.scalar_tensor_tensor(
                                        out=ctx[:, lt, sl], in0=vch[s2][:, lt, sl],
                                        scalar=att[:, s, s2, lt, r:r + 1],
                                        in1=ctx[:, lt, sl],
                                        op0=ALU.mult, op1=ALU.add)
                        for lt in range(LT):
                            for r in range(2):
                                rg = 2 * ci + r   # global r index
                                for fh in range(2):
                                    pst = ps_t.tile([128, 128], BF16, tag="pst", name="pst")
                                    nc.tensor.transpose(
                                        out=pst[:],
                                        in_=ctx[:, lt, r * F + fh * 128: r * F + (fh + 1) * 128],
                                        identity=identb[:])
                                    dst = rhs[:, s, fh, lt * 128:(lt + 1) * 128, rg]
                                    nc.any.tensor_copy(dst, pst[:])

                # ================= phase 2 =================
                for s in range(4):
                    for nb in range(NCHUNKS):
                        a0 = nb * 32
                        xt = xpool.tile([128, CC, CHUNK], BF16, tag="x", name="xt2")
                        nc.sync.dma_start(
                            out=xt[:],
                            in_=xs[s].ap().rearrange(
                                "p (c j n) -> p c j n", c=NCHUNKS, j=CC)[:, nb])
                        ot = outp.tile([128, CC, CHUNK], BF16, tag="ot", name="ot")
                        for j in range(CC):
                            pup = ps_up.tile([128, CHUNK], FP32, tag="up", name="pup")
                            for fh in range(2):
                                # contiguous [128, 32, 16] slice; cols already in
                                # output-pixel order n = a*16 + r
                                rap = rhs[:, s, fh, a0:a0 + 32, :]
                                nc.tensor.matmul(
                                    out=pup[:], lhsT=wu[:, s * 2 + fh, j, :],
                                    rhs=rap, start=(fh == 0), stop=False)
                            nc.tensor.matmul(
                                out=pup[:], lhsT=identb[:],
                                rhs=xt[:, j, :],
                                start=False, stop=True)
                            if j % 2 == 0:
                                nc.vector.tensor_scalar_add(
                                    out=ot[:, j, :], in0=pup[:],
                                    scalar1=bu[:, s * 4 + j:s * 4 + j + 1])
                            else:
                                nc.scalar.activation(
                                    out=ot[:, j, :], in_=pup[:], func=AF.Identity,
                                    bias=bu[:, s * 4 + j:s * 4 + j + 1], scale=1.0)
                        nc.sync.dma_start(
                            out=os_[s].ap().rearrange(
                                "p (c j n) -> p c j n", c=NCHUNKS, j=CC)[:, nb],
                            in_=ot[:])

            if loop_iters is None:
                _body()
            else:
                with tc.For_i(0, loop_iters, 1):
                    _body()

    nc.compile()
    return nc


def _prep_weights(inputs):
    """Fold BN into conv weights host-side; produce exact SBUF images."""
    import ml_dtypes
    f32 = np.float32
    bf16 = ml_dtypes.bfloat16
    g = {k: np.asarray(v, f32) for k, v in inputs.items()}
    sk = g["gk"] / np.sqrt(g["vk"] + EPS)            # [4, L]
    sq = g["gq"] / np.sqrt(g["vq"] + EPS)
    Wk_f = g["Wk"] * sk[:, :, None]                  # [4, L, C]
    Wq_f = g["Wq"] * sq[:, :, None]
    bk_f = (g["bk"] - g["mk"]) * sk + g["betak"]     # [4, L]
    bq_f = (g["bq"] - g["mq"]) * sq + g["betaq"]

    # wkq image [c_local, (s, mc), j, m]: lhsT chunks of [Wk_f|Wq_f]^T
    wkq = np.zeros((128, 16, 4, 128), f32)
    wdv = np.zeros((128, 8, 4, 128), f32)
    wuv = np.zeros((128, 8, 4, 128), f32)
    for s in range(4):
        Wcat = np.concatenate([Wk_f[s], Wq_f[s]], axis=0)  # [512 (kq-l), C]
        for mc in range(4):
            for j in range(CC):
                wkq[:, s * 4 + mc, j, :] = \
                    Wcat[mc * 128:(mc + 1) * 128, j * 128:(j + 1) * 128].T
        for mc in range(2):
            for j in range(CC):
                wdv[:, s * 2 + mc, j, :] = \
                    g["Wd"][s][mc * 128:(mc + 1) * 128, j * 128:(j + 1) * 128].T
        # wu: lhsT[f, c] = Wu[s].T ; [f_local, (s, fh), j, c_local]
        WuT = g["Wu"][s].T                           # [L=256 (f), C]
        for fh in range(2):
            for j in range(CC):
                wuv[:, s * 2 + fh, j, :] = \
                    WuT[fh * 128:(fh + 1) * 128, j * 128:(j + 1) * 128]

    bkq = np.zeros((128, 16), f32)
    bdv = np.zeros((128, 8), f32)
    buv = np.zeros((128, 16), f32)
    for s in range(4):
        for mc in range(4):
            src = bk_f[s] if mc < 2 else bq_f[s]
            bkq[:, s * 4 + mc] = src[(mc % 2) * 128:(mc % 2) * 128 + 128]
        for mc in range(2):
            bdv[:, s * 2 + mc] = g["bd"][s][mc * 128:(mc + 1) * 128]
        for j in range(CC):
            buv[:, s * 4 + j] = g["bu"][s][j * 128:(j + 1) * 128]
    ident = np.eye(128, dtype=f32)
    return {
        "wkq": wkq.reshape(128, -1).astype(bf16),
        "wd": wdv.reshape(128, -1).astype(bf16),
        "wu": wuv.reshape(128, -1).astype(bf16),
        "bkq": bkq, "bd": bdv, "bu": buv,
        "identb": ident.astype(bf16),
    }


def get_program(loop_iters=None):
    key = ("nc", loop_iters)
    if key not in _cached:
        _cached[key] = _build_program(loop_iters)
    return _cached[key]


def _x_image(xb):
    """[C, N] f32 -> [128, NCHUNKS*CC*CHUNK] bf16 DMA image (c = j*128 + p)."""
    import ml_dtypes
    x4 = np.asarray(xb, np.float32).reshape(CC, 128, NCHUNKS, CHUNK)
    return np.ascontiguousarray(
        x4.transpose(1, 2, 0, 3).reshape(128, -1)).astype(ml_dtypes.bfloat16)


def _o_unimage(o):
    """[128, NCHUNKS*CC*CHUNK] bf16 -> [C, HW, HW] f32."""
    o4 = np.asarray(o, np.float32).reshape(128, NCHUNKS, CC, CHUNK)
    return o4.transpose(2, 0, 1, 3).reshape(C, HW, HW)


def make_in_maps(inputs):
    w = _prep_weights(inputs)
    names = ("x_f", "x_g", "x_h", "x_t")
    xs = {nm: np.asarray(inputs[nm], np.float32).reshape(B, C, N) for nm in names}
    in_maps = []
    for b in range(B):
        m = dict(w)
        for s, nm in enumerate(names):
            m[f"x{s}"] = _x_image(xs[nm][b])
        in_maps.append(m)
    return in_maps


def kernel(**inputs):
    nc = get_program()
    in_maps = make_in_maps(inputs)
    res = run_bass_kernel_spmd(nc, in_maps, core_ids=list(range(NCORES)))
    outs = []
    for s in range(4):
        o = np.stack([_o_unimage(res.results[b][f"o{s}"]) for b in range(B)],
                     axis=0)
        outs.append(o)
    return tuple(outs)
